# revision 22
# baseline (speedup 1.0000x reference)
"""Trainium2 Bass kernel for nn_BIAS_1013612282247 (gnn_message_passing).

Computation (see reference):
  y[b,t,n,k]   = sum_m adj[k,n,m] * x[b,t,m]
  cheb         = relu(y @ cheb_w^T)                      # K -> HID
  sgc[b,o,n]   = sum_{t,j} cheb[b,t,n,j] gcn_w[o,t,0,j] + gcn_b[o]
  a/g          = dilated conv3(dil=2) over n of x halves + bias
  out          = a * sigmoid(g) + sgc                    # (B, F, N, 1)

Distribution: shard adj row-wise over nodes (N) across 8 cores; each core
computes its (B, F, 512) output slice; host concatenates.

Host-side marshalling: adj is pre-transposed to adjT[k, m, n] so the device
DMA-loads contraction-major tiles naturally (memory-bound problem: the
201 MB adj stream dominates; one extra host memcpy is free vs. on-chip
transposes). Matmuls run as float32r (fp32 rounded to 11-bit mantissa,
full-rate on the PE); operands are pre-rounded on the host so all loads go
through plain HWDGE DMAs.

PE-side packing: cheb processes two timesteps per matmul via a
block-diagonal [6, 128] stationary (f32r matmuls cannot use PE column
tile offsets, so both halves must come from one stationary); sgc then
contracts K=128 chunks; GLU packs a+g into one [36, 44] stationary.
"""
import sys

sys.path.insert(0, "/opt/trn_rl_repo")

from contextlib import ExitStack

import numpy as np

import concourse.bacc as bacc
import concourse.tile as tile
from concourse import mybir
from concourse.bass_utils import run_bass_kernel_spmd

f32 = mybir.dt.float32
f32r = mybir.dt.float32r

B, T, N, F, K, HID = 8, 12, 4096, 12, 3, 64
BT = B * T
NCORES = 8
NS = N // NCORES  # 512 nodes per core
MC = N // 128  # 32 contraction chunks

DT = f32r

Relu = mybir.ActivationFunctionType.Relu
Sigmoid = mybir.ActivationFunctionType.Sigmoid
Alu = mybir.AluOpType


def _round_f32r(a):
    """Round float32 array to f32r (round-nearest-even to 11-bit mantissa)."""
    bits = np.ascontiguousarray(a, np.float32).view(np.uint32)
    drop = 12
    half = np.uint32(1 << (drop - 1)) - np.uint32(1)
    lsb = (bits >> np.uint32(drop)) & np.uint32(1)
    out = (bits + half + lsb) & np.uint32((0xFFFFFFFF << drop) & 0xFFFFFFFF)
    return out.view(np.float32)


def _build():
    nc = bacc.Bacc("TRN2", target_bir_lowering=False, debug=False)

    adjP_d = nc.dram_tensor("adjP", (K, 8, 128, 2048), DT, kind="ExternalInput").ap()
    xt_d = nc.dram_tensor("xt", (128, MC * BT), DT, kind="ExternalInput").ap()
    xgp_d = nc.dram_tensor("xgp", (36, B * NS), DT, kind="ExternalInput").ap()
    chebT2_d = nc.dram_tensor("chebT2", (6, 128), DT, kind="ExternalInput").ap()
    gcnT_d = nc.dram_tensor("gcnT", (128, 6 * F), DT, kind="ExternalInput").ap()
    glupT_d = nc.dram_tensor("glupT", (36, 44), DT, kind="ExternalInput").ap()
    gcnb_d = nc.dram_tensor("gcnb", (F, 1), f32, kind="ExternalInput").ap()
    glu1b_d = nc.dram_tensor("glu1b", (F, 1), f32, kind="ExternalInput").ap()
    glu2b_d = nc.dram_tensor("glu2b", (F, 1), f32, kind="ExternalInput").ap()

    out_d = nc.dram_tensor("out", (B, F, NS), f32, kind="ExternalOutput").ap()
    # y bounce buffer, (q, c, tau, k, n): the per-(c, q) reload is one
    # contiguous [6, NS] block with (tau, k) on partitions, and the
    # bounce-out is one DMA per k (dest (c tau)=t merges to an affine dim)
    yd_d = nc.dram_tensor("yd", (B, 6, 2, K, NS), DT, kind="Internal").ap()

    with tile.TileContext(nc) as tc, ExitStack() as ctx:
        const = ctx.enter_context(tc.tile_pool(name="const", bufs=1))
        adjp = ctx.enter_context(tc.tile_pool(name="adjp", bufs=6))
        ysbp = ctx.enter_context(tc.tile_pool(name="ysbp", bufs=2))
        ytp = ctx.enter_context(tc.tile_pool(name="ytp", bufs=6))
        cp = ctx.enter_context(tc.tile_pool(name="cp", bufs=3))
        outp = ctx.enter_context(tc.tile_pool(name="outp", bufs=2))

        # ---- constant / small loads (all pre-rounded f32r on host) ----
        XT = const.tile([128, MC * BT], DT, tag="XT")
        nc.scalar.dma_start(XT[:], xt_d[:])
        chebT2 = const.tile([6, 128], DT, tag="chebT2")
        nc.sync.dma_start(chebT2[:], chebT2_d[:])
        gcnT = const.tile([128, 6 * F], DT, tag="gcnT")
        nc.sync.dma_start(gcnT[:], gcnT_d[:])
        glupT = const.tile([36, 44], DT, tag="glupT")
        nc.scalar.dma_start(glupT[:], glupT_d[:])
        xgp = const.tile([36, B * NS], DT, tag="xgp")
        nc.scalar.dma_start(xgp[:], xgp_d[:])
        gcnb = const.tile([F, 1], f32, tag="gcnb")
        nc.scalar.dma_start(gcnb[:], gcnb_d[:])
        glu1b = const.tile([F, 1], f32, tag="glu1b")
        nc.scalar.dma_start(glu1b[:], glu1b_d[:])
        glu2b = const.tile([F, 1], f32, tag="glu2b")
        nc.scalar.dma_start(glu2b[:], glu2b_d[:])

        glu_sb = const.tile([F, B * NS], f32, tag="glusb")

        # ---- phase A: y[k] = adj_k @ x -> psum [BT, NS], bounce to DRAM ----
        with tc.tile_pool(name="psy", bufs=1, space="PSUM") as psy:
            y_ps = [
                psy.tile([BT, NS], f32, tag=f"y{k}", name=f"y{k}") for k in range(K)
            ]
            for k in range(K):
                for blk in range(8):  # 4 contraction chunks per 1 MB DMA
                    at = adjp.tile([128, 4 * NS], DT, tag="adj")
                    nc.sync.dma_start(at[:], adjP_d[k, blk])
                    for c in range(4):
                        i = blk * 4 + c
                        nc.tensor.matmul(
                            y_ps[k][:],
                            XT[:, i * BT : (i + 1) * BT],
                            at[:, c * NS : (c + 1) * NS],
                            start=(i == 0),
                            stop=(i == MC - 1),
                        )
                y_sb = ysbp.tile([BT, NS], DT, tag="ysb")
                nc.vector.tensor_copy(y_sb[:], y_ps[k][:])
                # single bounce DMA: src partition p = t*8+b maps to dest
                # dims (t, b) since (c, tau) merge into an affine t stride
                nc.gpsimd.dma_start(
                    yd_d[:, :, :, k, :].rearrange("q c t n -> (c t) q n"),
                    y_sb[:],
                )

        # ---- GLU path (fills the PE gap between phase A and B/C) ----
        with tc.tile_pool(name="psg", bufs=2, space="PSUM") as psg:
            for q in range(B):
                ag_ps = psg.tile([44, NS], f32, tag="ag")
                nc.tensor.matmul(
                    ag_ps[:],
                    glupT[:],
                    xgp[:, q * NS : (q + 1) * NS],
                    start=True,
                    stop=True,
                )
                sig = outp.tile([F, NS], f32, tag="sig")
                nc.scalar.activation(sig[:], ag_ps[32:44, :], Sigmoid, bias=glu2b[:])
                nc.vector.scalar_tensor_tensor(
                    glu_sb[:, q * NS : (q + 1) * NS],
                    ag_ps[0:F, :],
                    glu1b[:],
                    sig[:],
                    Alu.add,
                    Alu.mult,
                )

        # ---- phase B/C: cheb (paired timesteps) + sgc (K=128 chunks) ----
        # Interleave 2-3 q streams so the PE always has independent matmuls
        # between a cheb and its dependent sgc (PE executes in order).
        with (
            tc.tile_pool(name="ps2", bufs=3, space="PSUM") as ps2,
            tc.tile_pool(name="psc", bufs=4, space="PSUM") as psc,
        ):
            for qs in ((0, 1, 2), (3, 4, 5), (6, 7)):
                sgc_ps = {}
                for q in qs:
                    sgc_ps[q] = ps2.tile([F, NS], f32, tag="sgc", name=f"sgc{q}")
                for c in range(6):
                    yqs = {}
                    for idx, q in enumerate(qs):
                        yq = ytp.tile([6, NS], DT, tag="yq", name="yq")
                        eng = nc.sync if idx % 2 == 0 else nc.scalar
                        eng.dma_start(yq[:], yd_d[q, c])
                        yqs[q] = yq
                    ch = {}
                    for q in qs:
                        ch_ps = psc.tile([128, NS], f32, tag="chp", name="chp")
                        nc.tensor.matmul(
                            ch_ps[:], chebT2[:], yqs[q][:], start=True, stop=True
                        )
                        ch[q] = ch_ps
                    cs = {}
                    for idx, q in enumerate(qs):
                        c_sb = cp.tile([128, NS], DT, tag="csb", name="csb")
                        if idx % 2 == 0:
                            nc.vector.tensor_scalar_max(c_sb[:], ch[q][:], 0.0)
                        else:
                            nc.scalar.activation(c_sb[:], ch[q][:], Relu)
                        cs[q] = c_sb
                    for q in qs:
                        nc.tensor.matmul(
                            sgc_ps[q][:],
                            gcnT[:, c * F : (c + 1) * F],
                            cs[q][:],
                            start=(c == 0),
                            stop=(c == 5),
                        )
                # ---- final: out = (sgc + gcn_b) + glu ----
                for q in qs:
                    o_sb = outp.tile([F, NS], f32, tag="osb", name="osb")
                    nc.vector.scalar_tensor_tensor(
                        o_sb[:],
                        sgc_ps[q][:],
                        gcnb[:],
                        glu_sb[:, q * NS : (q + 1) * NS],
                        Alu.add,
                        Alu.add,
                    )
                    nc.sync.dma_start(out_d[q], o_sb[:])

    nc.compile()
    return nc


_NC = None


def _get_nc():
    global _NC
    if _NC is None:
        _NC = _build()
    return _NC


def _prep_inputs(x, adj, cheb_w, gcn_w, gcn_b, glu1_w, glu1_b, glu2_w, glu2_b):
    x = np.asarray(x, np.float32)
    adj = np.asarray(adj, np.float32)
    X2 = x[..., 0]  # (B, T, N)
    # xt packed in SBUF tile layout: xt[p, i*BT + (t*8+b)] = x[b, t, i*128+p]
    xt = _round_f32r(
        np.ascontiguousarray(
            X2.transpose(2, 1, 0).reshape(MC, 128, BT).transpose(1, 0, 2)
        ).reshape(128, MC * BT)
    )
    xpad = np.pad(X2, ((0, 0), (0, 0), (2, 2)))
    cheb_w = np.asarray(cheb_w, np.float32)  # (HID, K)
    # chebT2: block-diagonal over two timesteps: rows (tau*3+k) -> out
    # partition (tau*64+j)
    chebT2 = np.zeros((6, 128), np.float32)
    chebT2[0:3, 0:64] = cheb_w.T
    chebT2[3:6, 64:128] = cheb_w.T
    # gcnT packed: [p, c*F + o] = gcn_w[o, t, 0, j] with c*128 + p = t*64 + j
    gcnT = (
        np.asarray(gcn_w, np.float32)[:, :, 0, :]
        .transpose(1, 2, 0)
        .reshape(6, 128, F)
        .transpose(1, 0, 2)
        .reshape(128, 6 * F)
    )
    # GLU packed stationary: rows 0:18 drive a (cols 0:12) from xg1 rows,
    # rows 18:36 drive g (cols 32:44) from xg2 rows
    glu1T = np.asarray(glu1_w, np.float32)[:, :, :, 1].transpose(1, 2, 0).reshape(18, F)
    glu2T = np.asarray(glu2_w, np.float32)[:, :, :, 1].transpose(1, 2, 0).reshape(18, F)
    glupT = np.zeros((36, 44), np.float32)
    glupT[0:18, 0:F] = glu1T
    glupT[18:36, 32 : 32 + F] = glu2T
    shared = {
        "xt": xt,
        "chebT2": _round_f32r(chebT2),
        "gcnT": _round_f32r(gcnT),
        "glupT": _round_f32r(glupT),
        "gcnb": np.asarray(gcn_b, np.float32).reshape(F, 1),
        "glu1b": np.asarray(glu1_b, np.float32).reshape(F, 1),
        "glu2b": np.asarray(glu2_b, np.float32).reshape(F, 1),
    }
    in_maps = []
    for core in range(NCORES):
        n0 = core * NS
        # xg[(c*3+tau), b, n] = xpad[b, t(c), n0 + n + 2*tau]
        xg = np.empty((36, B, NS), np.float32)
        for c in range(6):
            for tau in range(3):
                s = n0 + 2 * tau
                xg[c * 3 + tau] = xpad[:, c, s : s + NS]
                xg[18 + c * 3 + tau] = xpad[:, 6 + c, s : s + NS]
        # adjP[k, blk, p, c*NS + n] = adj[k, n0+n, blk*512 + c*128 + p]
        adjc = adj[:, n0 : n0 + NS, :].reshape(K, NS, 8, 4, 128)
        adjP = np.ascontiguousarray(adjc.transpose(0, 2, 4, 3, 1)).reshape(
            K, 8, 128, 4 * NS
        )
        in_maps.append(
            {
                "adjP": _round_f32r(adjP),
                "xgp": _round_f32r(xg.reshape(36, B * NS)),
                **shared,
            }
        )
    return in_maps


def kernel(
    x, adj, cheb_w, gcn_w, gcn_b, glu1_w, glu1_b, glu2_w, glu2_b, _profile=None
):
    nc = _get_nc()
    in_maps = _prep_inputs(
        x, adj, cheb_w, gcn_w, gcn_b, glu1_w, glu1_b, glu2_w, glu2_b
    )
    kwargs = dict(_profile) if _profile else {}
    res = run_bass_kernel_spmd(nc, in_maps, core_ids=list(range(NCORES)), **kwargs)
    outs = [res.results[c]["out"] for c in range(NCORES)]  # (B, F, NS) each
    full = np.concatenate(outs, axis=2)[..., None].astype(np.float32)
    if _profile is not None:
        return full, res
    return full


# revision 23
# speedup vs baseline: 1.0110x; 1.0110x over previous
"""Trainium2 Bass kernel for nn_BIAS_1013612282247 (gnn_message_passing).

Computation (see reference):
  y[b,t,n,k]   = sum_m adj[k,n,m] * x[b,t,m]
  cheb         = relu(y @ cheb_w^T)                      # K -> HID
  sgc[b,o,n]   = sum_{t,j} cheb[b,t,n,j] gcn_w[o,t,0,j] + gcn_b[o]
  a/g          = dilated conv3(dil=2) over n of x halves + bias
  out          = a * sigmoid(g) + sgc                    # (B, F, N, 1)

Distribution: shard adj row-wise over nodes (N) across 8 cores; each core
computes its (B, F, 512) output slice; host concatenates.

Host-side marshalling: adj is pre-transposed to adjT[k, m, n] so the device
DMA-loads contraction-major tiles naturally (memory-bound problem: the
201 MB adj stream dominates; one extra host memcpy is free vs. on-chip
transposes). Matmuls run as float32r (fp32 rounded to 11-bit mantissa,
full-rate on the PE); operands are pre-rounded on the host so all loads go
through plain HWDGE DMAs.

PE-side packing: cheb processes two timesteps per matmul via a
block-diagonal [6, 128] stationary (f32r matmuls cannot use PE column
tile offsets, so both halves must come from one stationary); sgc then
contracts K=128 chunks; GLU packs a+g into one [36, 44] stationary.
"""
import sys

sys.path.insert(0, "/opt/trn_rl_repo")

from contextlib import ExitStack

import numpy as np

import concourse.bacc as bacc
import concourse.tile as tile
from concourse import mybir
from concourse.bass_utils import run_bass_kernel_spmd

f32 = mybir.dt.float32
f32r = mybir.dt.float32r

B, T, N, F, K, HID = 8, 12, 4096, 12, 3, 64
BT = B * T
NCORES = 8
NS = N // NCORES  # 512 nodes per core
MC = N // 128  # 32 contraction chunks

DT = f32r

Relu = mybir.ActivationFunctionType.Relu
Sigmoid = mybir.ActivationFunctionType.Sigmoid
Alu = mybir.AluOpType


def _round_f32r(a):
    """Round float32 array to f32r (round-nearest-even to 11-bit mantissa)."""
    bits = np.ascontiguousarray(a, np.float32).view(np.uint32)
    drop = 12
    half = np.uint32(1 << (drop - 1)) - np.uint32(1)
    lsb = (bits >> np.uint32(drop)) & np.uint32(1)
    out = (bits + half + lsb) & np.uint32((0xFFFFFFFF << drop) & 0xFFFFFFFF)
    return out.view(np.float32)


def _build():
    nc = bacc.Bacc("TRN2", target_bir_lowering=False, debug=False)

    adjP_d = nc.dram_tensor("adjP", (K, 8, 128, 2048), DT, kind="ExternalInput").ap()
    xt_d = nc.dram_tensor("xt", (128, MC * BT), DT, kind="ExternalInput").ap()
    xgp_d = nc.dram_tensor("xgp", (36, B * NS), DT, kind="ExternalInput").ap()
    chebT2_d = nc.dram_tensor("chebT2", (6, 128), DT, kind="ExternalInput").ap()
    gcnT_d = nc.dram_tensor("gcnT", (128, 6 * F), DT, kind="ExternalInput").ap()
    glupT_d = nc.dram_tensor("glupT", (36, 44), DT, kind="ExternalInput").ap()
    gcnb_d = nc.dram_tensor("gcnb", (F, 1), f32, kind="ExternalInput").ap()
    glu1b_d = nc.dram_tensor("glu1b", (F, 1), f32, kind="ExternalInput").ap()
    glu2b_d = nc.dram_tensor("glu2b", (F, 1), f32, kind="ExternalInput").ap()

    out_d = nc.dram_tensor("out", (B, F, NS), f32, kind="ExternalOutput").ap()
    # y bounce buffer, (q, c, tau, k, n): the per-(c, q) reload is one
    # contiguous [6, NS] block with (tau, k) on partitions, and the
    # bounce-out is one DMA per k (dest (c tau)=t merges to an affine dim)
    yd_d = nc.dram_tensor("yd", (B, 6, 2, K, NS), DT, kind="Internal").ap()

    with tile.TileContext(nc) as tc, ExitStack() as ctx:
        const = ctx.enter_context(tc.tile_pool(name="const", bufs=1))
        adjp = ctx.enter_context(tc.tile_pool(name="adjp", bufs=6))
        ysbp = ctx.enter_context(tc.tile_pool(name="ysbp", bufs=2))
        ytp = ctx.enter_context(tc.tile_pool(name="ytp", bufs=10))
        cp = ctx.enter_context(tc.tile_pool(name="cp", bufs=4))
        outp = ctx.enter_context(tc.tile_pool(name="outp", bufs=2))

        # ---- constant / small loads (all pre-rounded f32r on host) ----
        XT = const.tile([128, MC * BT], DT, tag="XT")
        nc.scalar.dma_start(XT[:], xt_d[:])
        chebT2 = const.tile([6, 128], DT, tag="chebT2")
        nc.sync.dma_start(chebT2[:], chebT2_d[:])
        gcnT = const.tile([128, 6 * F], DT, tag="gcnT")
        nc.sync.dma_start(gcnT[:], gcnT_d[:])
        glupT = const.tile([36, 44], DT, tag="glupT")
        nc.scalar.dma_start(glupT[:], glupT_d[:])
        xgp = const.tile([36, B * NS], DT, tag="xgp")
        nc.scalar.dma_start(xgp[:], xgp_d[:])
        gcnb = const.tile([F, 1], f32, tag="gcnb")
        nc.scalar.dma_start(gcnb[:], gcnb_d[:])
        glu1b = const.tile([F, 1], f32, tag="glu1b")
        nc.scalar.dma_start(glu1b[:], glu1b_d[:])
        glu2b = const.tile([F, 1], f32, tag="glu2b")
        nc.scalar.dma_start(glu2b[:], glu2b_d[:])

        glu_sb = const.tile([F, B * NS], f32, tag="glusb")

        # ---- phase A: y[k] = adj_k @ x -> psum [BT, NS], bounce to DRAM ----
        with tc.tile_pool(name="psy", bufs=1, space="PSUM") as psy:
            y_ps = [
                psy.tile([BT, NS], f32, tag=f"y{k}", name=f"y{k}") for k in range(K)
            ]
            for k in range(K):
                for blk in range(8):  # 4 contraction chunks per 1 MB DMA
                    at = adjp.tile([128, 4 * NS], DT, tag="adj")
                    nc.sync.dma_start(at[:], adjP_d[k, blk])
                    for c in range(4):
                        i = blk * 4 + c
                        nc.tensor.matmul(
                            y_ps[k][:],
                            XT[:, i * BT : (i + 1) * BT],
                            at[:, c * NS : (c + 1) * NS],
                            start=(i == 0),
                            stop=(i == MC - 1),
                        )
                y_sb = ysbp.tile([BT, NS], DT, tag="ysb")
                nc.vector.tensor_copy(y_sb[:], y_ps[k][:])
                # single bounce DMA: src partition p = t*8+b maps to dest
                # dims (t, b) since (c, tau) merge into an affine t stride
                nc.gpsimd.dma_start(
                    yd_d[:, :, :, k, :].rearrange("q c t n -> (c t) q n"),
                    y_sb[:],
                )

        # ---- GLU path (fills the PE gap between phase A and B/C) ----
        with tc.tile_pool(name="psg", bufs=2, space="PSUM") as psg:
            for q in range(B):
                ag_ps = psg.tile([44, NS], f32, tag="ag")
                nc.tensor.matmul(
                    ag_ps[:],
                    glupT[:],
                    xgp[:, q * NS : (q + 1) * NS],
                    start=True,
                    stop=True,
                )
                sig = outp.tile([F, NS], f32, tag="sig")
                nc.scalar.activation(sig[:], ag_ps[32:44, :], Sigmoid, bias=glu2b[:])
                nc.vector.scalar_tensor_tensor(
                    glu_sb[:, q * NS : (q + 1) * NS],
                    ag_ps[0:F, :],
                    glu1b[:],
                    sig[:],
                    Alu.add,
                    Alu.mult,
                )

        # ---- phase B/C: cheb (paired timesteps) + sgc (K=128 chunks) ----
        # Software-pipelined with a one-iteration skew: the PE stream is
        # cheb(c) x3, sgc(c-1) x3, ... so no matmul ever waits on the relu
        # chain and the PE stays dense (HAM stays warm at 2.4 GHz).
        with (
            tc.tile_pool(name="ps2", bufs=3, space="PSUM") as ps2,
            tc.tile_pool(name="psc", bufs=5, space="PSUM") as psc,
        ):
            for qs in ((0, 1, 2), (3, 4, 5), (6, 7)):
                sgc_ps = {}
                for q in qs:
                    sgc_ps[q] = ps2.tile([F, NS], f32, tag="sgc", name=f"sgc{q}")
                prev_cs = None
                for c in range(6):
                    yqs = {}
                    for idx, q in enumerate(qs):
                        yq = ytp.tile([6, NS], DT, tag="yq", name="yq")
                        eng = nc.sync if idx % 2 == 0 else nc.scalar
                        eng.dma_start(yq[:], yd_d[q, c])
                        yqs[q] = yq
                    ch = {}
                    for q in qs:
                        ch_ps = psc.tile([128, NS], f32, tag="chp", name="chp")
                        nc.tensor.matmul(
                            ch_ps[:], chebT2[:], yqs[q][:], start=True, stop=True
                        )
                        ch[q] = ch_ps
                    if prev_cs is not None:
                        for q in qs:
                            nc.tensor.matmul(
                                sgc_ps[q][:],
                                gcnT[:, (c - 1) * F : c * F],
                                prev_cs[q][:],
                                start=(c - 1 == 0),
                                stop=False,
                            )
                    cs = {}
                    for idx, q in enumerate(qs):
                        c_sb = cp.tile([128, NS], DT, tag="csb", name="csb")
                        if idx % 2 == 0:
                            nc.vector.tensor_scalar_max(c_sb[:], ch[q][:], 0.0)
                        else:
                            nc.scalar.activation(c_sb[:], ch[q][:], Relu)
                        cs[q] = c_sb
                    prev_cs = cs
                for q in qs:
                    nc.tensor.matmul(
                        sgc_ps[q][:],
                        gcnT[:, 5 * F : 6 * F],
                        prev_cs[q][:],
                        start=False,
                        stop=True,
                    )
                # ---- final: out = (sgc + gcn_b) + glu ----
                for q in qs:
                    o_sb = outp.tile([F, NS], f32, tag="osb", name="osb")
                    nc.vector.scalar_tensor_tensor(
                        o_sb[:],
                        sgc_ps[q][:],
                        gcnb[:],
                        glu_sb[:, q * NS : (q + 1) * NS],
                        Alu.add,
                        Alu.add,
                    )
                    nc.sync.dma_start(out_d[q], o_sb[:])

    nc.compile()
    return nc


_NC = None


def _get_nc():
    global _NC
    if _NC is None:
        _NC = _build()
    return _NC


def _prep_inputs(x, adj, cheb_w, gcn_w, gcn_b, glu1_w, glu1_b, glu2_w, glu2_b):
    x = np.asarray(x, np.float32)
    adj = np.asarray(adj, np.float32)
    X2 = x[..., 0]  # (B, T, N)
    # xt packed in SBUF tile layout: xt[p, i*BT + (t*8+b)] = x[b, t, i*128+p]
    xt = _round_f32r(
        np.ascontiguousarray(
            X2.transpose(2, 1, 0).reshape(MC, 128, BT).transpose(1, 0, 2)
        ).reshape(128, MC * BT)
    )
    xpad = np.pad(X2, ((0, 0), (0, 0), (2, 2)))
    cheb_w = np.asarray(cheb_w, np.float32)  # (HID, K)
    # chebT2: block-diagonal over two timesteps: rows (tau*3+k) -> out
    # partition (tau*64+j)
    chebT2 = np.zeros((6, 128), np.float32)
    chebT2[0:3, 0:64] = cheb_w.T
    chebT2[3:6, 64:128] = cheb_w.T
    # gcnT packed: [p, c*F + o] = gcn_w[o, t, 0, j] with c*128 + p = t*64 + j
    gcnT = (
        np.asarray(gcn_w, np.float32)[:, :, 0, :]
        .transpose(1, 2, 0)
        .reshape(6, 128, F)
        .transpose(1, 0, 2)
        .reshape(128, 6 * F)
    )
    # GLU packed stationary: rows 0:18 drive a (cols 0:12) from xg1 rows,
    # rows 18:36 drive g (cols 32:44) from xg2 rows
    glu1T = np.asarray(glu1_w, np.float32)[:, :, :, 1].transpose(1, 2, 0).reshape(18, F)
    glu2T = np.asarray(glu2_w, np.float32)[:, :, :, 1].transpose(1, 2, 0).reshape(18, F)
    glupT = np.zeros((36, 44), np.float32)
    glupT[0:18, 0:F] = glu1T
    glupT[18:36, 32 : 32 + F] = glu2T
    shared = {
        "xt": xt,
        "chebT2": _round_f32r(chebT2),
        "gcnT": _round_f32r(gcnT),
        "glupT": _round_f32r(glupT),
        "gcnb": np.asarray(gcn_b, np.float32).reshape(F, 1),
        "glu1b": np.asarray(glu1_b, np.float32).reshape(F, 1),
        "glu2b": np.asarray(glu2_b, np.float32).reshape(F, 1),
    }
    in_maps = []
    for core in range(NCORES):
        n0 = core * NS
        # xg[(c*3+tau), b, n] = xpad[b, t(c), n0 + n + 2*tau]
        xg = np.empty((36, B, NS), np.float32)
        for c in range(6):
            for tau in range(3):
                s = n0 + 2 * tau
                xg[c * 3 + tau] = xpad[:, c, s : s + NS]
                xg[18 + c * 3 + tau] = xpad[:, 6 + c, s : s + NS]
        # adjP[k, blk, p, c*NS + n] = adj[k, n0+n, blk*512 + c*128 + p]
        adjc = adj[:, n0 : n0 + NS, :].reshape(K, NS, 8, 4, 128)
        adjP = np.ascontiguousarray(adjc.transpose(0, 2, 4, 3, 1)).reshape(
            K, 8, 128, 4 * NS
        )
        in_maps.append(
            {
                "adjP": _round_f32r(adjP),
                "xgp": _round_f32r(xg.reshape(36, B * NS)),
                **shared,
            }
        )
    return in_maps


def kernel(
    x, adj, cheb_w, gcn_w, gcn_b, glu1_w, glu1_b, glu2_w, glu2_b, _profile=None
):
    nc = _get_nc()
    in_maps = _prep_inputs(
        x, adj, cheb_w, gcn_w, gcn_b, glu1_w, glu1_b, glu2_w, glu2_b
    )
    kwargs = dict(_profile) if _profile else {}
    res = run_bass_kernel_spmd(nc, in_maps, core_ids=list(range(NCORES)), **kwargs)
    outs = [res.results[c]["out"] for c in range(NCORES)]  # (B, F, NS) each
    full = np.concatenate(outs, axis=2)[..., None].astype(np.float32)
    if _profile is not None:
        return full, res
    return full


# revision 25
# speedup vs baseline: 1.1781x; 1.1653x over previous
"""Trainium2 Bass kernel for nn_BIAS_1013612282247 (gnn_message_passing).

Computation (see reference):
  y[b,t,n,k]   = sum_m adj[k,n,m] * x[b,t,m]
  cheb         = relu(y @ cheb_w^T)                      # K -> HID
  sgc[b,o,n]   = sum_{t,j} cheb[b,t,n,j] gcn_w[o,t,0,j] + gcn_b[o]
  a/g          = dilated conv3(dil=2) over n of x halves + bias
  out          = a * sigmoid(g) + sgc                    # (B, F, N, 1)

Distribution: shard adj row-wise over nodes (N) across 8 cores; each core
computes its (B, F, 512) output slice; host concatenates.

Host-side marshalling: adj is pre-transposed to adjT[k, m, n] so the device
DMA-loads contraction-major tiles naturally (memory-bound problem: the
201 MB adj stream dominates; one extra host memcpy is free vs. on-chip
transposes). Matmuls run as float32r (fp32 rounded to 11-bit mantissa,
full-rate on the PE); operands are pre-rounded on the host so all loads go
through plain HWDGE DMAs.

PE-side packing: cheb processes two timesteps per matmul via a
block-diagonal [6, 128] stationary (f32r matmuls cannot use PE column
tile offsets, so both halves must come from one stationary); sgc then
contracts K=128 chunks; GLU packs a+g into one [36, 44] stationary.
"""
import sys

sys.path.insert(0, "/opt/trn_rl_repo")

from contextlib import ExitStack

import numpy as np

import concourse.bacc as bacc
import concourse.tile as tile
from concourse import mybir
from concourse.bass_utils import run_bass_kernel_spmd

f32 = mybir.dt.float32
f32r = mybir.dt.float32r

B, T, N, F, K, HID = 8, 12, 4096, 12, 3, 64
BT = B * T
NCORES = 8
NS = N // NCORES  # 512 nodes per core
MC = N // 128  # 32 contraction chunks

DT = f32r

Relu = mybir.ActivationFunctionType.Relu
Sigmoid = mybir.ActivationFunctionType.Sigmoid
Alu = mybir.AluOpType


def _round_f32r(a):
    """Round float32 array to f32r (round-nearest-even to 11-bit mantissa)."""
    bits = np.ascontiguousarray(a, np.float32).view(np.uint32)
    drop = 12
    half = np.uint32(1 << (drop - 1)) - np.uint32(1)
    lsb = (bits >> np.uint32(drop)) & np.uint32(1)
    out = (bits + half + lsb) & np.uint32((0xFFFFFFFF << drop) & 0xFFFFFFFF)
    return out.view(np.float32)


def _build():
    nc = bacc.Bacc("TRN2", target_bir_lowering=False, debug=False)

    adjP_d = nc.dram_tensor("adjP", (K, 8, 128, 2048), DT, kind="ExternalInput").ap()
    xt_d = nc.dram_tensor("xt", (128, MC * BT), DT, kind="ExternalInput").ap()
    xgp_d = nc.dram_tensor("xgp", (36, B * NS), DT, kind="ExternalInput").ap()
    chebT2_d = nc.dram_tensor("chebT2", (128, 128), DT, kind="ExternalInput").ap()
    gcnT_d = nc.dram_tensor("gcnT", (128, 6 * F), DT, kind="ExternalInput").ap()
    glupT_d = nc.dram_tensor("glupT", (128, 44), DT, kind="ExternalInput").ap()
    gcnb_d = nc.dram_tensor("gcnb", (F, 1), f32, kind="ExternalInput").ap()
    glu1b_d = nc.dram_tensor("glu1b", (F, 1), f32, kind="ExternalInput").ap()
    glu2b_d = nc.dram_tensor("glu2b", (F, 1), f32, kind="ExternalInput").ap()

    out_d = nc.dram_tensor("out", (B, F, NS), f32, kind="ExternalOutput").ap()
    # y bounce buffer, (q, c, tau, k, n): the per-(c, q) reload is one
    # contiguous [6, NS] block with (tau, k) on partitions, and the
    # bounce-out is one DMA per k (dest (c tau)=t merges to an affine dim)
    yd_d = nc.dram_tensor("yd", (B, 6, 2, K, NS), DT, kind="Internal").ap()

    with tile.TileContext(nc) as tc, ExitStack() as ctx:
        const = ctx.enter_context(tc.tile_pool(name="const", bufs=1))
        adjp = ctx.enter_context(tc.tile_pool(name="adjp", bufs=6))
        ysbp = ctx.enter_context(tc.tile_pool(name="ysbp", bufs=2))
        ytp = ctx.enter_context(tc.tile_pool(name="ytp", bufs=10))
        cp = ctx.enter_context(tc.tile_pool(name="cp", bufs=4))
        outp = ctx.enter_context(tc.tile_pool(name="outp", bufs=2))

        # ---- constant / small loads (all pre-rounded f32r on host) ----
        XT = const.tile([128, MC * BT], DT, tag="XT")
        nc.scalar.dma_start(XT[:], xt_d[:])
        chebT2 = const.tile([128, 128], DT, tag="chebT2")
        nc.sync.dma_start(chebT2[:], chebT2_d[:])
        gcnT = const.tile([128, 6 * F], DT, tag="gcnT")
        nc.sync.dma_start(gcnT[:], gcnT_d[:])
        glupT = const.tile([128, 44], DT, tag="glupT")
        nc.scalar.dma_start(glupT[:], glupT_d[:])
        # xgp zero-padded to 128 partitions so GLU matmuls use a full-K
        # contraction (small-K matmuls keep the PE HAM throttle engaged)
        xgp = const.tile([128, B * NS], DT, tag="xgp")
        nc.vector.memset(xgp[:].bitcast(f32), 0.0)
        nc.scalar.dma_start(xgp[0:36, :], xgp_d[:])
        gcnb = const.tile([F, 1], f32, tag="gcnb")
        nc.scalar.dma_start(gcnb[:], gcnb_d[:])
        glu1b = const.tile([F, 1], f32, tag="glu1b")
        nc.scalar.dma_start(glu1b[:], glu1b_d[:])
        glu2b = const.tile([F, 1], f32, tag="glu2b")
        nc.scalar.dma_start(glu2b[:], glu2b_d[:])

        glu_sb = const.tile([F, B * NS], f32, tag="glusb")

        # ---- phase A: y[k] = adj_k @ x -> psum [BT, NS], bounce to DRAM ----
        with tc.tile_pool(name="psy", bufs=1, space="PSUM") as psy:
            y_ps = [
                psy.tile([BT, NS], f32, tag=f"y{k}", name=f"y{k}") for k in range(K)
            ]
            for k in range(K):
                for blk in range(8):  # 4 contraction chunks per 1 MB DMA
                    at = adjp.tile([128, 4 * NS], DT, tag="adj")
                    nc.sync.dma_start(at[:], adjP_d[k, blk])
                    for c in range(4):
                        i = blk * 4 + c
                        nc.tensor.matmul(
                            y_ps[k][:],
                            XT[:, i * BT : (i + 1) * BT],
                            at[:, c * NS : (c + 1) * NS],
                            start=(i == 0),
                            stop=(i == MC - 1),
                        )
                y_sb = ysbp.tile([BT, NS], DT, tag="ysb")
                nc.vector.tensor_copy(y_sb[:], y_ps[k][:])
                # single bounce DMA: src partition p = t*8+b maps to dest
                # dims (t, b) since (c, tau) merge into an affine t stride
                nc.gpsimd.dma_start(
                    yd_d[:, :, :, k, :].rearrange("q c t n -> (c t) q n"),
                    y_sb[:],
                )

        # ---- GLU path (fills the PE gap between phase A and B/C) ----
        with tc.tile_pool(name="psg", bufs=2, space="PSUM") as psg:
            for q in range(B):
                ag_ps = psg.tile([44, NS], f32, tag="ag")
                nc.tensor.matmul(
                    ag_ps[:],
                    glupT[:],
                    xgp[:, q * NS : (q + 1) * NS],
                    start=True,
                    stop=True,
                )
                sig = outp.tile([F, NS], f32, tag="sig")
                nc.scalar.activation(sig[:], ag_ps[32:44, :], Sigmoid, bias=glu2b[:])
                nc.vector.scalar_tensor_tensor(
                    glu_sb[:, q * NS : (q + 1) * NS],
                    ag_ps[0:F, :],
                    glu1b[:],
                    sig[:],
                    Alu.add,
                    Alu.mult,
                )

        # ---- phase B/C: cheb (paired timesteps) + sgc (K=128 chunks) ----
        # Software-pipelined with a one-iteration skew: the PE stream is
        # cheb(c) x3, sgc(c-1) x3, ... so no matmul ever waits on the relu
        # chain and the PE stays dense (HAM stays warm at 2.4 GHz).
        # yq tiles are persistent [128, NS] buffers zeroed once: DMAs only
        # rewrite rows 0:6, keeping cheb contractions at K=128.
        yq_tiles = [
            const.tile([128, NS], DT, tag=f"yqt{i}", name=f"yqt{i}")
            for i in range(9)
        ]
        for i in range(9):
            nc.vector.memset(yq_tiles[i][:].bitcast(f32), 0.0)
        yq_ctr = [0]
        with (
            tc.tile_pool(name="ps2", bufs=3, space="PSUM") as ps2,
            tc.tile_pool(name="psc", bufs=5, space="PSUM") as psc,
        ):
            for qs in ((0, 1, 2), (3, 4, 5), (6, 7)):
                sgc_ps = {}
                for q in qs:
                    sgc_ps[q] = ps2.tile([F, NS], f32, tag="sgc", name=f"sgc{q}")
                prev_cs = None
                for c in range(6):
                    yqs = {}
                    for idx, q in enumerate(qs):
                        yq = yq_tiles[yq_ctr[0] % 9]
                        yq_ctr[0] += 1
                        eng = nc.sync if idx % 2 == 0 else nc.scalar
                        eng.dma_start(yq[0:6, :], yd_d[q, c])
                        yqs[q] = yq
                    ch = {}
                    for q in qs:
                        ch_ps = psc.tile([128, NS], f32, tag="chp", name="chp")
                        nc.tensor.matmul(
                            ch_ps[:], chebT2[:], yqs[q][:], start=True, stop=True
                        )
                        ch[q] = ch_ps
                    if prev_cs is not None:
                        for q in qs:
                            nc.tensor.matmul(
                                sgc_ps[q][:],
                                gcnT[:, (c - 1) * F : c * F],
                                prev_cs[q][:],
                                start=(c - 1 == 0),
                                stop=False,
                            )
                    cs = {}
                    for idx, q in enumerate(qs):
                        c_sb = cp.tile([128, NS], DT, tag="csb", name="csb")
                        if idx % 2 == 0:
                            nc.vector.tensor_scalar_max(c_sb[:], ch[q][:], 0.0)
                        else:
                            nc.scalar.activation(c_sb[:], ch[q][:], Relu)
                        cs[q] = c_sb
                    prev_cs = cs
                for q in qs:
                    nc.tensor.matmul(
                        sgc_ps[q][:],
                        gcnT[:, 5 * F : 6 * F],
                        prev_cs[q][:],
                        start=False,
                        stop=True,
                    )
                # ---- final: out = (sgc + gcn_b) + glu ----
                for q in qs:
                    o_sb = outp.tile([F, NS], f32, tag="osb", name="osb")
                    nc.vector.scalar_tensor_tensor(
                        o_sb[:],
                        sgc_ps[q][:],
                        gcnb[:],
                        glu_sb[:, q * NS : (q + 1) * NS],
                        Alu.add,
                        Alu.add,
                    )
                    nc.sync.dma_start(out_d[q], o_sb[:])

    nc.compile()
    return nc


_NC = None


def _get_nc():
    global _NC
    if _NC is None:
        _NC = _build()
    return _NC


def _prep_inputs(x, adj, cheb_w, gcn_w, gcn_b, glu1_w, glu1_b, glu2_w, glu2_b):
    x = np.asarray(x, np.float32)
    adj = np.asarray(adj, np.float32)
    X2 = x[..., 0]  # (B, T, N)
    # xt packed in SBUF tile layout: xt[p, i*BT + (t*8+b)] = x[b, t, i*128+p]
    xt = _round_f32r(
        np.ascontiguousarray(
            X2.transpose(2, 1, 0).reshape(MC, 128, BT).transpose(1, 0, 2)
        ).reshape(128, MC * BT)
    )
    xpad = np.pad(X2, ((0, 0), (0, 0), (2, 2)))
    cheb_w = np.asarray(cheb_w, np.float32)  # (HID, K)
    # chebT2: block-diagonal over two timesteps: rows (tau*3+k) -> out
    # partition (tau*64+j)
    chebT2 = np.zeros((128, 128), np.float32)
    chebT2[0:3, 0:64] = cheb_w.T
    chebT2[3:6, 64:128] = cheb_w.T
    # gcnT packed: [p, c*F + o] = gcn_w[o, t, 0, j] with c*128 + p = t*64 + j
    gcnT = (
        np.asarray(gcn_w, np.float32)[:, :, 0, :]
        .transpose(1, 2, 0)
        .reshape(6, 128, F)
        .transpose(1, 0, 2)
        .reshape(128, 6 * F)
    )
    # GLU packed stationary: rows 0:18 drive a (cols 0:12) from xg1 rows,
    # rows 18:36 drive g (cols 32:44) from xg2 rows
    glu1T = np.asarray(glu1_w, np.float32)[:, :, :, 1].transpose(1, 2, 0).reshape(18, F)
    glu2T = np.asarray(glu2_w, np.float32)[:, :, :, 1].transpose(1, 2, 0).reshape(18, F)
    glupT = np.zeros((128, 44), np.float32)
    glupT[0:18, 0:F] = glu1T
    glupT[18:36, 32 : 32 + F] = glu2T
    shared = {
        "xt": xt,
        "chebT2": _round_f32r(chebT2),
        "gcnT": _round_f32r(gcnT),
        "glupT": _round_f32r(glupT),
        "gcnb": np.asarray(gcn_b, np.float32).reshape(F, 1),
        "glu1b": np.asarray(glu1_b, np.float32).reshape(F, 1),
        "glu2b": np.asarray(glu2_b, np.float32).reshape(F, 1),
    }
    in_maps = []
    for core in range(NCORES):
        n0 = core * NS
        # xg[(c*3+tau), b, n] = xpad[b, t(c), n0 + n + 2*tau]
        xg = np.empty((36, B, NS), np.float32)
        for c in range(6):
            for tau in range(3):
                s = n0 + 2 * tau
                xg[c * 3 + tau] = xpad[:, c, s : s + NS]
                xg[18 + c * 3 + tau] = xpad[:, 6 + c, s : s + NS]
        # adjP[k, blk, p, c*NS + n] = adj[k, n0+n, blk*512 + c*128 + p]
        adjc = adj[:, n0 : n0 + NS, :].reshape(K, NS, 8, 4, 128)
        adjP = np.ascontiguousarray(adjc.transpose(0, 2, 4, 3, 1)).reshape(
            K, 8, 128, 4 * NS
        )
        in_maps.append(
            {
                "adjP": _round_f32r(adjP),
                "xgp": _round_f32r(xg.reshape(36, B * NS)),
                **shared,
            }
        )
    return in_maps


def kernel(
    x, adj, cheb_w, gcn_w, gcn_b, glu1_w, glu1_b, glu2_w, glu2_b, _profile=None
):
    nc = _get_nc()
    in_maps = _prep_inputs(
        x, adj, cheb_w, gcn_w, gcn_b, glu1_w, glu1_b, glu2_w, glu2_b
    )
    kwargs = dict(_profile) if _profile else {}
    res = run_bass_kernel_spmd(nc, in_maps, core_ids=list(range(NCORES)), **kwargs)
    outs = [res.results[c]["out"] for c in range(NCORES)]  # (B, F, NS) each
    full = np.concatenate(outs, axis=2)[..., None].astype(np.float32)
    if _profile is not None:
        return full, res
    return full
